# revision 66
# baseline (speedup 1.0000x reference)
"""Multi-head self-attention on 8 Trainium2 NeuronCores.

Tensor-parallel over heads: core c owns heads 2c, 2c+1 (128 of the 1024
hidden columns).  The host pre-transposes x to x^T [1024, 4096] bf16 and
the per-core weight slices to [p, k*c] layout so every DMA is contiguous
2KB-per-partition lines.  Stages:
  1. Q^T/K^T = (w.T @ x^T + b) in [d, token] layout (2 heads stacked on
     partitions: 0:64 head0, 64:128 head1).
  2. V^T likewise, then PE transposes into V_aug [token, 65-per-head]
     where column 64/129 = 1.0 (ones column -> softmax denominator falls
     out of P@V as accumulator row 64).
  3. Attention in 4 chunks of (batch, 1024 queries), software-pipelined:
     scores^T tiles = K^T.T @ Q^T (K=64 contraction; the two heads run
     concurrently in disjoint PE row groups), P^T = exp(S^T/8) on ScalarE
     (|S/8| < 3 so exp cannot overflow), and the previous chunk's P@V
     accumulation plus projection/WO back-work fill the PE while ScalarE
     (the bottleneck, ~1.1us per [128,1024] exp) streams.
  4. normalize: one copy pso->ostg (frees the PSUM bank fast), recip of
     the den row on a partition-0 tile (reciprocal_approx_fast is a
     custom-DVE op: PSUM or offset-partition inputs are undefined/crash),
     gpsimd partition_broadcast, one tensor_mul -> attnT (bf16).
  5. partial = attnT.T @ wo[128 rows of this core] -> HBM (bf16).
Host sums the 8 partials and adds bo.

Scheduling: ScalarE must never starve.  Warm-up matmuls + the exp
ACT-table load issue at t=0 with no DMA dependency (HAM un-throttles the
PE clock during the DMA window and the 2.7us table load is off the
critical path).  Only sync/scalar/gpsimd can issue DMAs; the critical
x^T[:, 0:1024] is split across all three queues (per-queue bandwidth
~130 B/ns; aggregate ~314) with the merged bias tensor avoiding
descriptor-gen serialization, and later waves queue strictly behind so
they cannot steal bandwidth from the critical slices.  pt pool needs 46
bufs (a (chunk c-1, tile tt) slot releases only at chunk c's second PV
half; fewer bufs stall the exp stream ~6us mid-chunk).  Each head's
scores psum slot is refilled during the OTHER head's exp.  Tail WO
alternates the psp/pse pools (4 matmuls in flight; 2 slots serialize at
~1.4us/mm through the stage copies) with eh1 staged on the otherwise
idle ScalarE.

Shapes hardcoded for x:[2,2048,1024], 16 heads, d_k=64.
"""

import numpy as np
import ml_dtypes

import concourse.bass as bass
import concourse.tile as tile
from concourse import bacc, mybir
from concourse.bass import ts
from concourse.bass_utils import run_bass_kernel_spmd

BF16 = mybir.dt.bfloat16
F32 = mybir.dt.float32
NPBF16 = ml_dtypes.bfloat16

B = 2
S = 2048
D = 1024
NT = B * S  # 4096 tokens
DK = 64
NCORES = 8
HPC = 2  # heads per core
SC = 1024  # attention s-chunk (exp op free size)

_CACHE = {}


def _build_nc():
    nc = bacc.Bacc("TRN2", target_bir_lowering=False, debug=False,
                   num_devices=NCORES)

    xT = nc.dram_tensor("xT", [D, NT], BF16, kind="ExternalInput").ap()
    # weights host-transposed to [p, k*128] so the DMA is contiguous
    wq = nc.dram_tensor("wq", [128, D], BF16, kind="ExternalInput").ap()
    wk = nc.dram_tensor("wk", [128, D], BF16, kind="ExternalInput").ap()
    wv = nc.dram_tensor("wv", [128, D], BF16, kind="ExternalInput").ap()
    bqkv = nc.dram_tensor("bqkv", [128, 3], F32, kind="ExternalInput").ap()
    wo = nc.dram_tensor("wo", [128, D], BF16, kind="ExternalInput").ap()
    out = nc.dram_tensor("out", [NT, D], BF16, kind="ExternalOutput").ap()

    with tile.TileContext(nc) as tc:
        _emit(nc, tc, xT, wq, wk, wv, bqkv, wo, out)
    nc.compile()
    return nc


def _emit(nc, tc, xT, wq, wk, wv, bqkv, wo, out):
    import contextlib
    ctx = contextlib.ExitStack()
    with ctx:
        consts = ctx.enter_context(tc.tile_pool(name="consts", bufs=1))
        # 46 pt bufs (fewer cost ~4.6us/buf of chunk-boundary exp
        # stalls) AND 3-deep staging pools (2-deep gated the tail WO on
        # the DMA-read + 900ns sem-prop recycle), funded by the bf16
        # normalize staging below.
        ptp = ctx.enter_context(tc.tile_pool(name="ptp", bufs=46))
        # PSUM budget (8 banks): 3 scores slots (6 banks) for the paired
        # h0/h1 score bursts, 1 PV accumulator bank (quarter-groups), 1
        # extras bank (proj / V-proj / V-transpose / WO staging).
        psp = ctx.enter_context(tc.tile_pool(name="psp", bufs=3, space="PSUM"))
        pvp = ctx.enter_context(tc.tile_pool(name="pvp", bufs=1, space="PSUM"))
        pse = ctx.enter_context(tc.tile_pool(name="pse", bufs=1, space="PSUM"))
        stg = ctx.enter_context(tc.tile_pool(name="stg", bufs=3))
        nrm = ctx.enter_context(tc.tile_pool(name="nrm", bufs=2))
        vstg = ctx.enter_context(tc.tile_pool(name="vstg", bufs=3))

        # ---- persistent SBUF tensors ----
        xT_sb = consts.tile([128, 8, NT], BF16)      # 8 k-tiles of x^T
        wq_sb = consts.tile([128, 8, 128], BF16)
        wk_sb = consts.tile([128, 8, 128], BF16)
        wv_sb = consts.tile([128, 8, 128], BF16)
        bqkv_sb = consts.tile([128, 3], F32)
        bq_sb = bqkv_sb[:, 0:1]
        bk_sb = bqkv_sb[:, 1:2]
        bv_sb = bqkv_sb[:, 2:3]
        wo_sb = consts.tile([128, D], BF16)
        QT = consts.tile([128, NT], BF16)
        KT = consts.tile([128, NT], BF16)
        V_sb = consts.tile([128, 32, 130], BF16)     # [t-in-tile, t_tile, col]
        attnT = consts.tile([128, NT], BF16)
        ident = consts.tile([128, 128], BF16)
        wup = consts.tile([128, 128], BF16)

        # ---- t=0: PE warm-up + ACT table load, no DMA dependencies.
        # The 64 warm-up matmuls are load-bearing for the whole kernel's
        # clock state: with only 28, every exp on ACT ran 27% slower
        # (1335ns vs 1114ns per [128,1024] tile) for the entire run. ----
        nc.vector.memset(wup, 0.125)
        wups = pse.tile([128, 512], F32, tag="pse", name="wups")
        for i in range(64):
            nc.tensor.matmul(wups[:, 0:128], lhsT=wup, rhs=wup,
                             start=True, stop=True)
        tblw = stg.tile([128, 1], F32, tag="tblw", name="tblw")
        nc.scalar.activation(out=tblw, in_=wup[:, 0:1],
                             func=mybir.ActivationFunctionType.Exp)
        from concourse.masks import make_identity
        make_identity(nc, ident)

        # ---- DMAs: weights first on the gpsimd queue; x^T in merged
        # multi-k-tile waves (one dma_start covers all 8 k-tiles of a
        # column range -- 8x fewer descriptor-gen instructions).  The
        # scalar (ACT) queue carries exactly one wave so the exp stream
        # is never blocked behind descriptor generation. ----
        xT_p = xT.rearrange("(k p) n -> p k n", p=128)

        nc.sync.dma_start(out=xT_sb[:, 0:3, 0:1024], in_=xT_p[:, 0:3, 0:1024])
        nc.scalar.dma_start(out=xT_sb[:, 3:6, 0:1024],
                            in_=xT_p[:, 3:6, 0:1024])
        nc.gpsimd.dma_start(out=wq_sb, in_=wq.rearrange("p (k c) -> p k c", k=8))
        nc.gpsimd.dma_start(out=xT_sb[:, 6:8, 0:1024],
                            in_=xT_p[:, 6:8, 0:1024])
        nc.gpsimd.dma_start(out=wk_sb, in_=wk.rearrange("p (k c) -> p k c", k=8))
        nc.gpsimd.dma_start(out=bqkv_sb, in_=bqkv)
        nc.scalar.dma_start(out=wv_sb, in_=wv.rearrange("p (k c) -> p k c", k=8))
        nc.sync.dma_start(out=xT_sb[:, :, 1024:1536],
                          in_=xT_p[:, :, 1024:1536])
        nc.gpsimd.dma_start(out=xT_sb[:, :, 1536:2048],
                            in_=xT_p[:, :, 1536:2048])
        nc.sync.dma_start(out=xT_sb[:, :, 2048:3072],
                          in_=xT_p[:, :, 2048:3072])
        nc.gpsimd.dma_start(out=wo_sb, in_=wo)
        nc.gpsimd.dma_start(out=xT_sb[:, :, 3072:4096],
                            in_=xT_p[:, :, 3072:4096])

        # ones columns of V_aug (never touched by the per-tile copies)
        nc.vector.memset(V_sb[:, :, 64:65], 1.0)
        nc.vector.memset(V_sb[:, :, 129:130], 1.0)

        # ---- emit helpers ----
        vt_stage = {}

        def emit_v_proj(c, pool=None):
            # V^T chunk: [c128, 512 tokens] += wv[k].T @ xT[k] (+bias, ->bf16)
            # During chunk 0 the PV bank is idle, so the V pipeline
            # (proj + transposes) runs there; later chunks use pse.
            psv = (pool or pse).tile([128, 512], F32,
                                     tag="pv" if pool else "pse")
            for k in range(8):
                nc.tensor.matmul(psv, lhsT=wv_sb[:, k, :],
                                 rhs=xT_sb[:, k, ts(c, 512)],
                                 start=(k == 0), stop=(k == 7))
            vt = vstg.tile([128, 512], BF16, tag="vt", name=f"vt{c}")
            nc.vector.tensor_scalar_add(vt, psv, bv_sb)
            vt_stage[c] = vt

        def emit_v_tr(tt, pool=None):
            # transpose one 128x128 block of V^T into V_aug [t, col] layout
            c, j = divmod(tt, 4)
            trp = (pool or pse).tile([128, 128], BF16,
                                     tag="pv" if pool else "pse",
                                     name=f"trp{tt}")
            nc.tensor.transpose(trp, vt_stage[c][:, ts(j, 128)], ident)
            nc.vector.tensor_copy(V_sb[:, tt, 0:64], trp[:, 0:64])
            nc.vector.tensor_copy(V_sb[:, tt, 65:129], trp[:, 64:128])

        def emit_wo_tile(tt, use_act=False):
            for eh in range(2):
                pw = pse.tile([128, 512], F32, tag="pse")
                nc.tensor.matmul(pw, lhsT=attnT[:, ts(tt, 128)],
                                 rhs=wo_sb[:, ts(eh, 512)],
                                 start=True, stop=True)
                ob = stg.tile([128, 512], BF16, tag="ob")
                if use_act and eh == 1:
                    nc.scalar.activation(
                        out=ob, in_=pw,
                        func=mybir.ActivationFunctionType.Copy, bias=0.0)
                else:
                    nc.vector.tensor_copy(ob, pw)
                nc.sync.dma_start(
                    out=out[tt * 128:(tt + 1) * 128, eh * 512:(eh + 1) * 512],
                    in_=ob)

        def emit_proj_chunk(w_sb, b_sb, o_sb, n, w=512):
            # w-token chunk n (units of w) of the Q^T or K^T projection
            ps = pse.tile([128, 512], F32, tag="pse")
            for k in range(8):
                nc.tensor.matmul(ps[:, 0:w], lhsT=w_sb[:, k, :],
                                 rhs=xT_sb[:, k, ts(n, w)],
                                 start=(k == 0), stop=(k == 7))
            nc.vector.tensor_scalar_add(o_sb[:, ts(n, w)], ps[:, 0:w], b_sb)

        def emit_normalize_q(prev, h, c):
            # stage the accumulator out of PSUM first (one copy frees the
            # bank for the next PV quarter).  reciprocal_approx_fast is a
            # custom-DVE op: it needs a partition-0 SBUF input (PSUM or
            # offset-64 inputs produce garbage / hw crashes).
            b, sc, pts, pv_state = prev
            s0 = b * S + sc * SC + c * 512
            pso = pv_state['pso']
            # ostg/recb in bf16 (the fp32 island is only the reciprocal:
            # reciprocal_approx_* requires fp32 in AND out, and
            # partition_broadcast requires matching dtypes, so rsum stays
            # f32 and rsumb converts before the broadcast).  This plus
            # the spare SBUF funds the 46th pt buffer (each pt buffer is
            # worth ~4.6us of chunk-boundary exp stalls); costs ~0.7%
            # rel error, well inside the 2e-2 budget.
            ostg = nrm.tile([65, 512], BF16, tag="ostg", bufs=1)
            nc.vector.tensor_copy(ostg, pso[0:65, :])
            rsum = nrm.tile([1, 512], F32, tag="rsum", bufs=1)
            nc.vector.tensor_copy(rsum, ostg[64:65, :])
            nc.vector.reciprocal_approx_fast(out=rsum, in_=rsum)
            rsumb = nrm.tile([1, 512], BF16, tag="rsumb", bufs=1)
            nc.vector.tensor_copy(rsumb, rsum)
            recb = nrm.tile([64, 512], BF16, tag="recb", bufs=1)
            nc.gpsimd.partition_broadcast(recb, rsumb)
            nc.vector.tensor_mul(
                attnT[h * DK:(h + 1) * DK, s0:s0 + 512],
                ostg[0:64, :], recb)

        # PV runs in quarter-groups so ONE psum bank serves all of P@V:
        # quarter q accumulates (head, half) = QORDER[q] over all 16
        # t-tiles (4 per step), then normalizes, freeing the bank for the
        # next quarter (the ostg copy is the only read of the bank).
        QORDER = [(0, 0), (1, 0), (0, 1), (1, 1)]

        def emit_pv_step(prev, s):
            b, sc, pts, pv_state = prev
            q, r = divmod(s, 4)
            h, c = QORDER[q]
            if r == 0:
                pv_state['pso'] = pvp.tile([128, 512], F32, tag="pv",
                                           name=f"pso{b}_{sc}_{q}")
            pso = pv_state['pso']
            for dt in range(4):
                tt = 4 * r + dt
                nc.tensor.matmul(
                    pso[0:65, :],
                    lhsT=V_sb[:, b * 16 + tt, h * 65:(h + 1) * 65],
                    rhs=pts[tt][h][:, ts(c, 512)],
                    start=(tt == 0), stop=(tt == 15))
            if r == 3:
                emit_normalize_q(prev, h, c)

        # ---- prologue: QT[0:1024] and KT[0:256], k-loops interleaved in
        # DMA-arrival order (k0/k3/k6 land first, then k1/k4/k7, then
        # k2/k5) so each matmul starts the moment its x^T slice lands.
        psQ = psp.tile([128, SC], F32, tag="ps", name="psQ")
        psK = psp.tile([128, SC], F32, tag="ps", name="psK")
        for k in range(8):
            nc.tensor.matmul(psQ[:, 0:512], lhsT=wq_sb[:, k, :],
                             rhs=xT_sb[:, k, 0:512],
                             start=(k == 0), stop=(k == 7))
        for k in range(8):
            nc.tensor.matmul(psK[:, 0:512], lhsT=wk_sb[:, k, :],
                             rhs=xT_sb[:, k, 0:512],
                             start=(k == 0), stop=(k == 7))
        nc.vector.tensor_scalar_add(KT[:, 0:512], psK[:, 0:512], bk_sb)
        for k in range(8):
            nc.tensor.matmul(psQ[:, 512:1024], lhsT=wq_sb[:, k, :],
                             rhs=xT_sb[:, k, 512:1024],
                             start=(k == 0), stop=(k == 7))
        nc.vector.tensor_scalar_add(QT[:, 0:512], psQ[:, 0:512], bq_sb)
        nc.vector.tensor_scalar_add(QT[:, 512:1024], psQ[:, 512:1024], bq_sb)

        # deferred PE work, interleaved into the ACT-bound attention loop.
        qk = [(wq_sb, bq_sb, QT), (wk_sb, bk_sb, KT)]

        def pj(which, n256):
            return lambda: emit_proj_chunk(*qk[which], n256, w=256)

        def vp(c, pool=None):
            f = lambda: emit_v_proj(c, pool)  # noqa: E731
            f.big = True  # too long to sit between the PV step and burst
            return f

        def vt(t, pool=None):
            return lambda: emit_v_tr(t, pool)

        def wot(t, use_act=False):
            return lambda: emit_wo_tile(t, use_act)

        extras_per_chunk = [
            # chunk 0 (b0,sc0): no PV yet, so the V pipeline (proj 0-3 +
            # tr 0-15, needed complete by chunk-1 step 3) runs on the
            # idle PV bank while KT 2-7 (JIT for the score bursts) and
            # KT 8-15 + QT 4-7 (chunk 1 queries) run on the extras bank.
            [(0, vp(0, pvp)), (0, pj(1, 2)), (1, vt(0, pvp)),
             (1, pj(1, 3)), (2, vt(1, pvp)), (2, pj(1, 4)),
             (3, vt(2, pvp)), (3, pj(1, 5)), (4, vt(3, pvp)),
             (4, vp(1, pvp)), (5, vt(4, pvp)), (5, pj(1, 6)),
             (6, vt(5, pvp)), (6, pj(1, 7)), (7, vt(6, pvp)),
             (7, pj(0, 4)), (8, vt(7, pvp)), (8, vp(2, pvp)),
             (9, vt(8, pvp)), (9, pj(0, 5)), (10, vt(9, pvp)),
             (10, pj(0, 6)), (11, vt(10, pvp)), (11, pj(0, 7)),
             (12, vt(11, pvp)), (12, vp(3, pvp)), (13, vt(12, pvp)),
             (13, vt(13, pvp)), (14, vt(14, pvp)), (14, vt(15, pvp))],
            # chunk 1 (b0,sc1): KT 8-15 (b1 keys, ahead of chunk-2
            # bursts), QT 8-11 (chunk 2 queries)
            [(0, pj(1, 8)), (2, pj(1, 9)), (3, pj(0, 8)), (4, pj(1, 10)),
             (5, pj(1, 11)), (6, pj(0, 9)), (7, pj(1, 12)),
             (8, pj(1, 13)), (9, pj(0, 10)), (10, pj(1, 14)),
             (11, pj(1, 15)), (12, pj(0, 11))],
            # chunk 2 (b1,sc0): V proj 4-7 + tr 16-31 (b1, ALL needed by
            # chunk-3 PV quarter 0 steps 0-3), QT 12-15 (chunk 3)
            [(0, vp(4)), (1, vt(16)), (1, vt(17)), (2, pj(0, 12)),
             (3, vt(18)), (3, vt(19)), (4, vp(5)), (5, vt(20)),
             (5, vt(21)), (6, pj(0, 13)), (7, vt(22)), (7, vt(23)),
             (8, vp(6)), (9, vt(24)), (9, vt(25)), (10, pj(0, 14)),
             (11, vt(26)), (11, vt(27)), (12, vp(7)), (13, vt(28)),
             (13, vt(29)), (14, pj(0, 15)), (15, vt(30)), (15, vt(31))],
            # chunk 3 (b1,sc1): WO 0-15 (chunk 0-1 tokens, attnT complete
            # since end of chunks 1/2)
            [(0, wot(0)), (1, wot(1)), (2, wot(2)), (3, wot(3)),
             (4, wot(4)), (5, wot(5)), (6, wot(6)), (7, wot(7)),
             (8, wot(8)), (9, wot(9)), (10, wot(10)), (11, wot(11)),
             (12, wot(12)), (13, wot(13)), (14, wot(14)), (15, wot(15))],
        ]

        def emit_scores_burst(b, sc, tt):
            # both heads' next score tiles as ONE interleaved burst
            # [h0n0, h1n0, h0n1, h1n1]: adjacent matmuls sit in disjoint
            # PE row groups (KT head slices at partitions 0-63 / 64-127),
            # which the PE executes concurrently (~145ns each vs 250ns,
            # measured).  Slot allocation order (h0 then h1) matches
            # exp-read order; with 3 psp slots the h1 slot frees at
            # exp(h0) completion ~1.13us into the window, so the burst
            # is emitted after the PV step + one small extra.
            s0 = b * S + sc * SC
            k0 = b * S + tt * 128
            pss = [psp.tile([128, SC], F32, tag="ps",
                            name=f"ps{b}_{sc}_{tt}_{h}") for h in range(HPC)]
            # h1 leads each pair: if the PE arrives before exp(h0) frees
            # the h1 slot, h1n0 parks in the wait queue while h0n0
            # bypasses and streams -- and h1n0 launches concurrently the
            # moment its dependency resolves mid-stream.  Arriving late,
            # adjacent h1/h0 pairs overlap as usual.  Either way the row
            # groups interleave, which a lagging h1 under the old
            # h0-first order never achieved.
            for n2 in range(2):
                for h in (1, 0):
                    hsl = slice(h * DK, (h + 1) * DK)
                    nc.tensor.matmul(
                        pss[h][:, ts(n2, 512)],
                        lhsT=KT[hsl, k0:k0 + 128],
                        rhs=QT[hsl, s0 + n2 * 512:s0 + (n2 + 1) * 512],
                        start=True, stop=True)
            return pss

        chunks = [(b, sc) for b in range(B) for sc in range(S // SC)]
        prev = None
        pair = emit_scores_burst(0, 0, 0)
        for ci, (b, sc) in enumerate(chunks):
            extras = sorted(extras_per_chunk[ci], key=lambda e: e[0])
            pts = []
            cur = (b, sc, pts, {})
            for tt in range(16):
                # next tile to prefill (crossing chunk boundaries)
                if tt + 1 < 16:
                    nxt = (b, sc, tt + 1)
                elif ci + 1 < len(chunks):
                    nxt = (*chunks[ci + 1], 0)
                else:
                    nxt = None
                row = []
                for h in range(HPC):
                    pt = ptp.tile([128, SC], BF16, tag="pt")
                    if ci == 0 and tt == 0:
                        for n2 in range(2):
                            nc.scalar.activation(
                                out=pt[:, ts(n2, 512)],
                                in_=pair[h][:, ts(n2, 512)],
                                func=mybir.ActivationFunctionType.Exp,
                                scale=0.125)
                    else:
                        nc.scalar.activation(
                            out=pt, in_=pair[h],
                            func=mybir.ActivationFunctionType.Exp,
                            scale=0.125)
                    row.append(pt)
                pts.append(row)
                # PV + one small extra fill the PE past exp(h0)'s
                # completion (~1.13us), so when the PE reaches the burst
                # the h1 slot (freed by exp(h0)) is already writable and
                # the h0/h1 pairs execute concurrently.  vp extras are
                # too long (~1.7us) -- they would push the burst past the
                # next exp's deadline, so they stay after the burst.
                if prev is not None:
                    emit_pv_step(prev, tt)
                if (extras and extras[0][0] <= tt
                        and not getattr(extras[0][1], 'big', False)):
                    extras.pop(0)[1]()
                if nxt is not None:
                    pair = emit_scores_burst(*nxt)
                while extras and extras[0][0] <= tt:
                    extras.pop(0)[1]()
            for _, e in extras:
                e()
            prev = cur
        # tail: PV quarters + normalize for the last chunk; WO through
        # the now-free scores pool.
        def emit_wo_tail(tt):
            # BOTH halves allocate separate tiles from the freed 3-deep
            # scores pool.  The single pse bank is one long serial chain
            # (16 chunk-3 WO extras + tail eh1 slots, ~1.5us per link:
            # mm + copy + sem props) -- threading ANY tail work through
            # it was the 1.8us/tile serializer in v5/v6/v7/v10/v11.
            # psp keeps ~3 halves in flight and never touches that
            # chain.  eh0 stages on DVE, eh1 on the now-idle ACT; DMA
            # issues: eh0 on gpsimd, eh1 on sync.
            for eh in range(2):
                pwt = psp.tile([128, SC], F32, tag="ps",
                               name=f"pw{tt}_{eh}")
                pw = pwt[:, 0:512]
                nc.tensor.matmul(pw, lhsT=attnT[:, ts(tt, 128)],
                                 rhs=wo_sb[:, ts(eh, 512)],
                                 start=True, stop=True)
                if eh == 1:
                    ob = vstg.tile([128, 512], BF16, tag="vt",
                                   name=f"obt{tt}")
                    nc.scalar.activation(
                        out=ob, in_=pw,
                        func=mybir.ActivationFunctionType.Copy, bias=0.0)
                else:
                    ob = stg.tile([128, 512], BF16, tag="ob")
                    nc.vector.tensor_copy(ob, pw)
                q = nc.gpsimd if eh == 0 else nc.sync
                q.dma_start(
                    out=out[tt * 128:(tt + 1) * 128, eh * 512:(eh + 1) * 512],
                    in_=ob)

        # c3-half0 tiles (24-27) unlock after the h1,half0 normalize at
        # s=7; half1 tiles (28-31) after the final normalize at s=15.
        for s in range(16):
            emit_pv_step(prev, s)
            if s < 8:
                emit_wo_tail(16 + s)
            elif s in (9, 11, 13, 15):
                emit_wo_tail(24 + (s - 9) // 2)
        for tt in range(28, 32):
            emit_wo_tail(tt)


def _prep_in_maps(x, wq, bq, wk, bk, wv, bv, wo):
    x2 = np.asarray(x, np.float32).reshape(NT, D)
    xT = np.ascontiguousarray(x2.T).astype(NPBF16)
    wq = np.asarray(wq, np.float32)
    wk = np.asarray(wk, np.float32)
    wv = np.asarray(wv, np.float32)
    wo = np.asarray(wo, np.float32)
    bq = np.asarray(bq, np.float32)
    bk = np.asarray(bk, np.float32)
    bv = np.asarray(bv, np.float32)

    def wslice(w, cs):
        # [1024, 128] core slice -> [p, k*c] = [128, 1024] contiguous
        wt = w[:, cs].reshape(8, 128, 128).transpose(1, 0, 2)
        return np.ascontiguousarray(wt.reshape(128, D)).astype(NPBF16)

    in_maps = []
    for c in range(NCORES):
        cs = slice(c * 128, (c + 1) * 128)
        in_maps.append({
            "xT": xT,
            "wq": wslice(wq, cs),
            "wk": wslice(wk, cs),
            "wv": wslice(wv, cs),
            "bqkv": np.ascontiguousarray(
                np.stack([bq[cs], bk[cs], bv[cs]], axis=1)),
            "wo": wo[cs, :].astype(NPBF16),
        })
    return in_maps


def kernel(x, wq, bq, wk, bk, wv, bv, wo, bo, _run_kwargs=None):
    if "nc" not in _CACHE:
        _CACHE["nc"] = _build_nc()
    nc = _CACHE["nc"]
    in_maps = _prep_in_maps(x, wq, bq, wk, bk, wv, bv, wo)
    res = run_bass_kernel_spmd(nc, in_maps, list(range(NCORES)),
                               **(_run_kwargs or {}))
    acc = np.zeros((NT, D), np.float32)
    for c in range(NCORES):
        acc += res.results[c]["out"].astype(np.float32)
    acc += np.asarray(bo, np.float32)[None, :]
    if _run_kwargs:
        _CACHE["last_results"] = res
    return acc.reshape(B, S, D)



# revision 69
# speedup vs baseline: 1.0117x; 1.0117x over previous
"""Multi-head self-attention on 8 Trainium2 NeuronCores.

Tensor-parallel over heads: core c owns heads 2c, 2c+1 (128 of the 1024
hidden columns).  The host pre-transposes x to x^T [1024, 4096] bf16 and
the per-core weight slices to [p, k*c] layout so every DMA is contiguous
2KB-per-partition lines.  Stages:
  1. Q^T/K^T = (w.T @ x^T + b) in [d, token] layout (2 heads stacked on
     partitions: 0:64 head0, 64:128 head1).
  2. V^T likewise, then PE transposes into V_aug [token, 65-per-head]
     where column 64/129 = 1.0 (ones column -> softmax denominator falls
     out of P@V as accumulator row 64).
  3. Attention in 4 chunks of (batch, 1024 queries), software-pipelined:
     scores^T tiles = K^T.T @ Q^T (K=64 contraction; the two heads run
     concurrently in disjoint PE row groups), P^T = exp(S^T/8) on ScalarE
     (|S/8| < 3 so exp cannot overflow), and the previous chunk's P@V
     accumulation plus projection/WO back-work fill the PE while ScalarE
     (the bottleneck, ~1.1us per [128,1024] exp) streams.
  4. normalize: one copy pso->ostg (frees the PSUM bank fast), recip of
     the den row on a partition-0 tile (reciprocal_approx_fast is a
     custom-DVE op: PSUM or offset-partition inputs are undefined/crash),
     gpsimd partition_broadcast, one tensor_mul -> attnT (bf16).
  5. partial = attnT.T @ wo[128 rows of this core] -> HBM (bf16).
Host sums the 8 partials and adds bo.

Scheduling: ScalarE must never starve.  Warm-up matmuls + the exp
ACT-table load issue at t=0 with no DMA dependency (HAM un-throttles the
PE clock during the DMA window and the 2.7us table load is off the
critical path).  Only sync/scalar/gpsimd can issue DMAs; the critical
x^T[:, 0:1024] is split across all three queues (per-queue bandwidth
~130 B/ns; aggregate ~314) with the merged bias tensor avoiding
descriptor-gen serialization, and later waves queue strictly behind so
they cannot steal bandwidth from the critical slices.  pt pool needs 46
bufs (a (chunk c-1, tile tt) slot releases only at chunk c's second PV
half; fewer bufs stall the exp stream ~6us mid-chunk).  Each head's
scores psum slot is refilled during the OTHER head's exp.  Tail WO
alternates the psp/pse pools (4 matmuls in flight; 2 slots serialize at
~1.4us/mm through the stage copies) with eh1 staged on the otherwise
idle ScalarE.

Shapes hardcoded for x:[2,2048,1024], 16 heads, d_k=64.
"""

import numpy as np
import ml_dtypes

import concourse.bass as bass
import concourse.tile as tile
from concourse import bacc, mybir
from concourse.bass import ts
from concourse.bass_utils import run_bass_kernel_spmd

BF16 = mybir.dt.bfloat16
F32 = mybir.dt.float32
NPBF16 = ml_dtypes.bfloat16

B = 2
S = 2048
D = 1024
NT = B * S  # 4096 tokens
DK = 64
NCORES = 8
HPC = 2  # heads per core
SC = 1024  # attention s-chunk (exp op free size)

_CACHE = {}


def _build_nc():
    nc = bacc.Bacc("TRN2", target_bir_lowering=False, debug=False,
                   num_devices=NCORES)

    xT = nc.dram_tensor("xT", [D, NT], BF16, kind="ExternalInput").ap()
    # weights host-transposed to [p, k*128] so the DMA is contiguous
    wq = nc.dram_tensor("wq", [128, D], BF16, kind="ExternalInput").ap()
    wk = nc.dram_tensor("wk", [128, D], BF16, kind="ExternalInput").ap()
    wv = nc.dram_tensor("wv", [128, D], BF16, kind="ExternalInput").ap()
    bqkv = nc.dram_tensor("bqkv", [128, 3], F32, kind="ExternalInput").ap()
    wo = nc.dram_tensor("wo", [128, D], BF16, kind="ExternalInput").ap()
    out = nc.dram_tensor("out", [NT, D], BF16, kind="ExternalOutput").ap()

    with tile.TileContext(nc) as tc:
        _emit(nc, tc, xT, wq, wk, wv, bqkv, wo, out)
    nc.compile()
    return nc


def _emit(nc, tc, xT, wq, wk, wv, bqkv, wo, out):
    import contextlib
    ctx = contextlib.ExitStack()
    with ctx:
        consts = ctx.enter_context(tc.tile_pool(name="consts", bufs=1))
        # 46 pt bufs (fewer cost ~4.6us/buf of chunk-boundary exp
        # stalls) AND 3-deep staging pools (2-deep gated the tail WO on
        # the DMA-read + 900ns sem-prop recycle), funded by the bf16
        # normalize staging below.
        ptp = ctx.enter_context(tc.tile_pool(name="ptp", bufs=46))
        # PSUM budget (8 banks): 3 scores slots (6 banks) for the paired
        # h0/h1 score bursts, 1 PV accumulator bank (quarter-groups), 1
        # extras bank (proj / V-proj / V-transpose / WO staging).
        psp = ctx.enter_context(tc.tile_pool(name="psp", bufs=3, space="PSUM"))
        pvp = ctx.enter_context(tc.tile_pool(name="pvp", bufs=1, space="PSUM"))
        pse = ctx.enter_context(tc.tile_pool(name="pse", bufs=1, space="PSUM"))
        stg = ctx.enter_context(tc.tile_pool(name="stg", bufs=3))
        nrm = ctx.enter_context(tc.tile_pool(name="nrm", bufs=2))
        vstg = ctx.enter_context(tc.tile_pool(name="vstg", bufs=3))

        # ---- persistent SBUF tensors ----
        xT_sb = consts.tile([128, 8, NT], BF16)      # 8 k-tiles of x^T
        wq_sb = consts.tile([128, 8, 128], BF16)
        wk_sb = consts.tile([128, 8, 128], BF16)
        wv_sb = consts.tile([128, 8, 128], BF16)
        bqkv_sb = consts.tile([128, 3], F32)
        bq_sb = bqkv_sb[:, 0:1]
        bk_sb = bqkv_sb[:, 1:2]
        bv_sb = bqkv_sb[:, 2:3]
        wo_sb = consts.tile([128, D], BF16)
        QT = consts.tile([128, NT], BF16)
        KT = consts.tile([128, NT], BF16)
        V_sb = consts.tile([128, 32, 130], BF16)     # [t-in-tile, t_tile, col]
        attnT = consts.tile([128, NT], BF16)
        ident = consts.tile([128, 128], BF16)
        wup = consts.tile([128, 128], BF16)

        # ---- t=0: PE warm-up + ACT table load, no DMA dependencies.
        # The 64 warm-up matmuls are load-bearing for the whole kernel's
        # clock state: with only 28, every exp on ACT ran 27% slower
        # (1335ns vs 1114ns per [128,1024] tile) for the entire run. ----
        nc.vector.memset(wup, 0.125)
        wups = pse.tile([128, 512], F32, tag="pse", name="wups")
        for i in range(64):
            nc.tensor.matmul(wups[:, 0:128], lhsT=wup, rhs=wup,
                             start=True, stop=True)
        tblw = stg.tile([128, 1], F32, tag="tblw", name="tblw")
        nc.scalar.activation(out=tblw, in_=wup[:, 0:1],
                             func=mybir.ActivationFunctionType.Exp)
        from concourse.masks import make_identity
        make_identity(nc, ident)

        # ---- DMAs: weights first on the gpsimd queue; x^T in merged
        # multi-k-tile waves (one dma_start covers all 8 k-tiles of a
        # column range -- 8x fewer descriptor-gen instructions).  The
        # scalar (ACT) queue carries exactly one wave so the exp stream
        # is never blocked behind descriptor generation. ----
        xT_p = xT.rearrange("(k p) n -> p k n", p=128)

        nc.sync.dma_start(out=xT_sb[:, 0:3, 0:1024], in_=xT_p[:, 0:3, 0:1024])
        nc.scalar.dma_start(out=xT_sb[:, 3:6, 0:1024],
                            in_=xT_p[:, 3:6, 0:1024])
        nc.gpsimd.dma_start(out=wq_sb, in_=wq.rearrange("p (k c) -> p k c", k=8))
        nc.gpsimd.dma_start(out=xT_sb[:, 6:8, 0:1024],
                            in_=xT_p[:, 6:8, 0:1024])
        nc.gpsimd.dma_start(out=wk_sb, in_=wk.rearrange("p (k c) -> p k c", k=8))
        nc.gpsimd.dma_start(out=bqkv_sb, in_=bqkv)
        nc.scalar.dma_start(out=wv_sb, in_=wv.rearrange("p (k c) -> p k c", k=8))
        nc.sync.dma_start(out=xT_sb[:, :, 1024:1536],
                          in_=xT_p[:, :, 1024:1536])
        nc.gpsimd.dma_start(out=xT_sb[:, :, 1536:2048],
                            in_=xT_p[:, :, 1536:2048])
        nc.sync.dma_start(out=xT_sb[:, :, 2048:3072],
                          in_=xT_p[:, :, 2048:3072])
        nc.gpsimd.dma_start(out=wo_sb, in_=wo)
        nc.gpsimd.dma_start(out=xT_sb[:, :, 3072:4096],
                            in_=xT_p[:, :, 3072:4096])

        # ones columns of V_aug (never touched by the per-tile copies)
        nc.vector.memset(V_sb[:, :, 64:65], 1.0)
        nc.vector.memset(V_sb[:, :, 129:130], 1.0)

        # ---- emit helpers ----
        vt_stage = {}

        def emit_v_proj(c, pool=None):
            # V^T chunk: [c128, 512 tokens] += wv[k].T @ xT[k] (+bias, ->bf16)
            # During chunk 0 the PV bank is idle, so the V pipeline
            # (proj + transposes) runs there; later chunks use pse.
            psv = (pool or pse).tile([128, 512], F32,
                                     tag="pv" if pool else "pse")
            for k in range(8):
                nc.tensor.matmul(psv, lhsT=wv_sb[:, k, :],
                                 rhs=xT_sb[:, k, ts(c, 512)],
                                 start=(k == 0), stop=(k == 7))
            vt = vstg.tile([128, 512], BF16, tag="vt", name=f"vt{c}")
            nc.vector.tensor_scalar_add(vt, psv, bv_sb)
            vt_stage[c] = vt

        def emit_v_tr(tt, pool=None):
            # transpose one 128x128 block of V^T into V_aug [t, col] layout
            c, j = divmod(tt, 4)
            trp = (pool or pse).tile([128, 128], BF16,
                                     tag="pv" if pool else "pse",
                                     name=f"trp{tt}")
            nc.tensor.transpose(trp, vt_stage[c][:, ts(j, 128)], ident)
            nc.vector.tensor_copy(V_sb[:, tt, 0:64], trp[:, 0:64])
            nc.vector.tensor_copy(V_sb[:, tt, 65:129], trp[:, 64:128])

        def emit_wo_tile(tt, use_act=False):
            for eh in range(2):
                pw = pse.tile([128, 512], F32, tag="pse")
                nc.tensor.matmul(pw, lhsT=attnT[:, ts(tt, 128)],
                                 rhs=wo_sb[:, ts(eh, 512)],
                                 start=True, stop=True)
                ob = stg.tile([128, 512], BF16, tag="ob")
                if use_act and eh == 1:
                    nc.scalar.activation(
                        out=ob, in_=pw,
                        func=mybir.ActivationFunctionType.Copy, bias=0.0)
                else:
                    nc.vector.tensor_copy(ob, pw)
                nc.sync.dma_start(
                    out=out[tt * 128:(tt + 1) * 128, eh * 512:(eh + 1) * 512],
                    in_=ob)

        def emit_proj_chunk(w_sb, b_sb, o_sb, n, w=512):
            # w-token chunk n (units of w) of the Q^T or K^T projection
            ps = pse.tile([128, 512], F32, tag="pse")
            for k in range(8):
                nc.tensor.matmul(ps[:, 0:w], lhsT=w_sb[:, k, :],
                                 rhs=xT_sb[:, k, ts(n, w)],
                                 start=(k == 0), stop=(k == 7))
            nc.vector.tensor_scalar_add(o_sb[:, ts(n, w)], ps[:, 0:w], b_sb)

        def emit_normalize_q(prev, h, c):
            # stage the accumulator out of PSUM first (one copy frees the
            # bank for the next PV quarter).  reciprocal_approx_fast is a
            # custom-DVE op: it needs a partition-0 SBUF input (PSUM or
            # offset-64 inputs produce garbage / hw crashes).
            b, sc, pts, pv_state = prev
            s0 = b * S + sc * SC + c * 512
            pso = pv_state['pso']
            # ostg/recb in bf16 (the fp32 island is only the reciprocal:
            # reciprocal_approx_* requires fp32 in AND out, and
            # partition_broadcast requires matching dtypes, so rsum stays
            # f32 and rsumb converts before the broadcast).  This plus
            # the spare SBUF funds the 46th pt buffer (each pt buffer is
            # worth ~4.6us of chunk-boundary exp stalls); costs ~0.7%
            # rel error, well inside the 2e-2 budget.
            ostg = nrm.tile([65, 512], BF16, tag="ostg", bufs=1)
            nc.vector.tensor_copy(ostg, pso[0:65, :])
            rsum = nrm.tile([1, 512], F32, tag="rsum", bufs=1)
            nc.vector.tensor_copy(rsum, ostg[64:65, :])
            nc.vector.reciprocal_approx_fast(out=rsum, in_=rsum)
            rsumb = nrm.tile([1, 512], BF16, tag="rsumb", bufs=1)
            nc.vector.tensor_copy(rsumb, rsum)
            recb = nrm.tile([64, 512], BF16, tag="recb", bufs=1)
            nc.gpsimd.partition_broadcast(recb, rsumb)
            nc.vector.tensor_mul(
                attnT[h * DK:(h + 1) * DK, s0:s0 + 512],
                ostg[0:64, :], recb)

        # PV runs in quarter-groups so ONE psum bank serves all of P@V:
        # quarter q accumulates (head, half) = QORDER[q] over all 16
        # t-tiles (4 per step), then normalizes, freeing the bank for the
        # next quarter (the ostg copy is the only read of the bank).
        QORDER = [(0, 0), (1, 0), (0, 1), (1, 1)]

        def emit_pv_step(prev, s):
            b, sc, pts, pv_state = prev
            q, r = divmod(s, 4)
            h, c = QORDER[q]
            if r == 0:
                pv_state['pso'] = pvp.tile([128, 512], F32, tag="pv",
                                           name=f"pso{b}_{sc}_{q}")
            pso = pv_state['pso']
            for dt in range(4):
                tt = 4 * r + dt
                nc.tensor.matmul(
                    pso[0:65, :],
                    lhsT=V_sb[:, b * 16 + tt, h * 65:(h + 1) * 65],
                    rhs=pts[tt][h][:, ts(c, 512)],
                    start=(tt == 0), stop=(tt == 15))
            if r == 3:
                emit_normalize_q(prev, h, c)

        # ---- prologue: QT[0:1024] and KT[0:256], k-loops interleaved in
        # DMA-arrival order (k0/k3/k6 land first, then k1/k4/k7, then
        # k2/k5) so each matmul starts the moment its x^T slice lands.
        psQ = psp.tile([128, SC], F32, tag="ps", name="psQ")
        psK = psp.tile([128, SC], F32, tag="ps", name="psK")
        for k in range(8):
            nc.tensor.matmul(psQ[:, 0:512], lhsT=wq_sb[:, k, :],
                             rhs=xT_sb[:, k, 0:512],
                             start=(k == 0), stop=(k == 7))
        for k in range(8):
            nc.tensor.matmul(psK[:, 0:512], lhsT=wk_sb[:, k, :],
                             rhs=xT_sb[:, k, 0:512],
                             start=(k == 0), stop=(k == 7))
        nc.vector.tensor_scalar_add(KT[:, 0:512], psK[:, 0:512], bk_sb)
        for k in range(8):
            nc.tensor.matmul(psQ[:, 512:1024], lhsT=wq_sb[:, k, :],
                             rhs=xT_sb[:, k, 512:1024],
                             start=(k == 0), stop=(k == 7))
        nc.vector.tensor_scalar_add(QT[:, 0:512], psQ[:, 0:512], bq_sb)
        nc.vector.tensor_scalar_add(QT[:, 512:1024], psQ[:, 512:1024], bq_sb)

        # deferred PE work, interleaved into the ACT-bound attention loop.
        qk = [(wq_sb, bq_sb, QT), (wk_sb, bk_sb, KT)]

        def pj(which, n256):
            return lambda: emit_proj_chunk(*qk[which], n256, w=256)

        def vp(c, pool=None):
            f = lambda: emit_v_proj(c, pool)  # noqa: E731
            f.big = True  # too long to sit between the PV step and burst
            return f

        def vt(t, pool=None):
            return lambda: emit_v_tr(t, pool)

        def wot(t, use_act=False):
            return lambda: emit_wo_tile(t, use_act)

        extras_per_chunk = [
            # chunk 0 (b0,sc0): no PV yet, so the V pipeline (proj 0-3 +
            # tr 0-15, needed complete by chunk-1 step 3) runs on the
            # idle PV bank while KT 2-7 (JIT for the score bursts) and
            # KT 8-15 + QT 4-7 (chunk 1 queries) run on the extras bank.
            [(0, vp(0, pvp)), (0, pj(1, 2)), (1, vt(0, pvp)),
             (1, pj(1, 3)), (2, vt(1, pvp)), (2, pj(1, 4)),
             (3, vt(2, pvp)), (3, pj(1, 5)), (4, vt(3, pvp)),
             (4, vp(1, pvp)), (5, vt(4, pvp)), (5, pj(1, 6)),
             (6, vt(5, pvp)), (6, pj(1, 7)), (7, vt(6, pvp)),
             (7, pj(0, 4)), (8, vt(7, pvp)), (8, vp(2, pvp)),
             (9, vt(8, pvp)), (9, pj(0, 5)), (10, vt(9, pvp)),
             (10, pj(0, 6)), (11, vt(10, pvp)), (11, pj(0, 7)),
             (12, vt(11, pvp)), (12, vp(3, pvp)), (13, vt(12, pvp)),
             (13, vt(13, pvp)), (14, vt(14, pvp)), (14, vt(15, pvp))],
            # chunk 1 (b0,sc1): KT 8-15 (b1 keys, ahead of chunk-2
            # bursts), QT 8-15 (chunk 2 AND chunk 3 queries -- QT 12-15
            # moved here from chunk 2 so every c1 step has a small
            # pre-burst filler and every c2 step can lead with a vt)
            [(0, pj(1, 8)), (1, pj(0, 12)), (2, pj(1, 9)), (3, pj(0, 8)),
             (4, pj(1, 10)), (5, pj(1, 11)), (6, pj(0, 9)),
             (7, pj(1, 12)), (8, pj(1, 13)), (9, pj(0, 10)),
             (10, pj(1, 14)), (11, pj(1, 15)), (12, pj(0, 11)),
             (13, pj(0, 13)), (14, pj(0, 14)), (15, pj(0, 15))],
            # chunk 2 (b1,sc0): V proj 4-7 + tr 16-31 (b1, ALL needed by
            # chunk-3 PV quarter 0 steps 0-3); vts lead each step so the
            # pre-burst pop never skips on a big vp
            [(0, vp(4)), (1, vt(16)), (2, vt(17)), (3, vt(18)),
             (4, vt(19)), (4, vp(5)), (5, vt(20)), (6, vt(21)),
             (7, vt(22)), (8, vt(23)), (8, vp(6)), (9, vt(24)),
             (10, vt(25)), (11, vt(26)), (12, vt(27)), (12, vp(7)),
             (13, vt(28)), (14, vt(29)), (15, vt(30)), (15, vt(31))],
            # chunk 3 (b1,sc1): WO 0-15 (chunk 0-1 tokens, attnT complete
            # since end of chunks 1/2)
            [(0, wot(0)), (1, wot(1)), (2, wot(2)), (3, wot(3)),
             (4, wot(4)), (5, wot(5)), (6, wot(6)), (7, wot(7)),
             (8, wot(8)), (9, wot(9)), (10, wot(10)), (11, wot(11)),
             (12, wot(12)), (13, wot(13)), (14, wot(14)), (15, wot(15))],
        ]

        def emit_scores_burst(b, sc, tt):
            # both heads' next score tiles as ONE interleaved burst
            # [h0n0, h1n0, h0n1, h1n1]: adjacent matmuls sit in disjoint
            # PE row groups (KT head slices at partitions 0-63 / 64-127),
            # which the PE executes concurrently (~145ns each vs 250ns,
            # measured).  Slot allocation order (h0 then h1) matches
            # exp-read order; with 3 psp slots the h1 slot frees at
            # exp(h0) completion ~1.13us into the window, so the burst
            # is emitted after the PV step + one small extra.
            s0 = b * S + sc * SC
            k0 = b * S + tt * 128
            pss = [psp.tile([128, SC], F32, tag="ps",
                            name=f"ps{b}_{sc}_{tt}_{h}") for h in range(HPC)]
            for n2 in range(2):
                for h in range(HPC):
                    hsl = slice(h * DK, (h + 1) * DK)
                    nc.tensor.matmul(
                        pss[h][:, ts(n2, 512)],
                        lhsT=KT[hsl, k0:k0 + 128],
                        rhs=QT[hsl, s0 + n2 * 512:s0 + (n2 + 1) * 512],
                        start=True, stop=True)
            return pss

        chunks = [(b, sc) for b in range(B) for sc in range(S // SC)]
        prev = None
        pair = emit_scores_burst(0, 0, 0)
        for ci, (b, sc) in enumerate(chunks):
            extras = sorted(extras_per_chunk[ci], key=lambda e: e[0])
            pts = []
            cur = (b, sc, pts, {})
            for tt in range(16):
                # next tile to prefill (crossing chunk boundaries)
                if tt + 1 < 16:
                    nxt = (b, sc, tt + 1)
                elif ci + 1 < len(chunks):
                    nxt = (*chunks[ci + 1], 0)
                else:
                    nxt = None
                row = []
                for h in range(HPC):
                    pt = ptp.tile([128, SC], BF16, tag="pt")
                    if ci == 0 and tt == 0:
                        for n2 in range(2):
                            nc.scalar.activation(
                                out=pt[:, ts(n2, 512)],
                                in_=pair[h][:, ts(n2, 512)],
                                func=mybir.ActivationFunctionType.Exp,
                                scale=0.125)
                    else:
                        nc.scalar.activation(
                            out=pt, in_=pair[h],
                            func=mybir.ActivationFunctionType.Exp,
                            scale=0.125)
                    row.append(pt)
                pts.append(row)
                # PV + one small extra fill the PE past exp(h0)'s
                # completion (~1.13us), so when the PE reaches the burst
                # the h1 slot (freed by exp(h0)) is already writable and
                # the h0/h1 pairs execute concurrently.  vp extras are
                # too long (~1.7us) -- they would push the burst past the
                # next exp's deadline, so they stay after the burst.
                if prev is not None:
                    emit_pv_step(prev, tt)
                # chunk 0 has no PV, so two small extras bridge the PE to
                # the h1 slot's release (exp(h0) end, ~1.13us in).
                for _ in range(1 if prev is not None else 2):
                    if (extras and extras[0][0] <= tt
                            and not getattr(extras[0][1], 'big', False)):
                        extras.pop(0)[1]()
                if nxt is not None:
                    pair = emit_scores_burst(*nxt)
                while extras and extras[0][0] <= tt:
                    extras.pop(0)[1]()
            for _, e in extras:
                e()
            prev = cur
        # tail: PV quarters + normalize for the last chunk; WO through
        # the now-free scores pool.
        def emit_wo_tail(tt):
            # BOTH halves allocate separate tiles from the freed 3-deep
            # scores pool.  The single pse bank is one long serial chain
            # (16 chunk-3 WO extras + tail eh1 slots, ~1.5us per link:
            # mm + copy + sem props) -- threading ANY tail work through
            # it was the 1.8us/tile serializer in v5/v6/v7/v10/v11.
            # psp keeps ~3 halves in flight and never touches that
            # chain.  eh0 stages on DVE, eh1 on the now-idle ACT; DMA
            # issues: eh0 on gpsimd, eh1 on sync.
            for eh in range(2):
                pwt = psp.tile([128, SC], F32, tag="ps",
                               name=f"pw{tt}_{eh}")
                pw = pwt[:, 0:512]
                nc.tensor.matmul(pw, lhsT=attnT[:, ts(tt, 128)],
                                 rhs=wo_sb[:, ts(eh, 512)],
                                 start=True, stop=True)
                if eh == 1:
                    ob = vstg.tile([128, 512], BF16, tag="vt",
                                   name=f"obt{tt}")
                    nc.scalar.activation(
                        out=ob, in_=pw,
                        func=mybir.ActivationFunctionType.Copy, bias=0.0)
                else:
                    ob = stg.tile([128, 512], BF16, tag="ob")
                    nc.vector.tensor_copy(ob, pw)
                q = nc.gpsimd if eh == 0 else nc.sync
                q.dma_start(
                    out=out[tt * 128:(tt + 1) * 128, eh * 512:(eh + 1) * 512],
                    in_=ob)

        # c3-half0 tiles (24-27) unlock after the h1,half0 normalize at
        # s=7; half1 tiles (28-31) after the final normalize at s=15.
        for s in range(16):
            emit_pv_step(prev, s)
            if s < 8:
                emit_wo_tail(16 + s)
            elif s in (9, 11, 13, 15):
                emit_wo_tail(24 + (s - 9) // 2)
        for tt in range(28, 32):
            emit_wo_tail(tt)


def _prep_in_maps(x, wq, bq, wk, bk, wv, bv, wo):
    x2 = np.asarray(x, np.float32).reshape(NT, D)
    xT = np.ascontiguousarray(x2.T).astype(NPBF16)
    wq = np.asarray(wq, np.float32)
    wk = np.asarray(wk, np.float32)
    wv = np.asarray(wv, np.float32)
    wo = np.asarray(wo, np.float32)
    bq = np.asarray(bq, np.float32)
    bk = np.asarray(bk, np.float32)
    bv = np.asarray(bv, np.float32)

    def wslice(w, cs):
        # [1024, 128] core slice -> [p, k*c] = [128, 1024] contiguous
        wt = w[:, cs].reshape(8, 128, 128).transpose(1, 0, 2)
        return np.ascontiguousarray(wt.reshape(128, D)).astype(NPBF16)

    in_maps = []
    for c in range(NCORES):
        cs = slice(c * 128, (c + 1) * 128)
        in_maps.append({
            "xT": xT,
            "wq": wslice(wq, cs),
            "wk": wslice(wk, cs),
            "wv": wslice(wv, cs),
            "bqkv": np.ascontiguousarray(
                np.stack([bq[cs], bk[cs], bv[cs]], axis=1)),
            "wo": wo[cs, :].astype(NPBF16),
        })
    return in_maps


def kernel(x, wq, bq, wk, bk, wv, bv, wo, bo, _run_kwargs=None):
    if "nc" not in _CACHE:
        _CACHE["nc"] = _build_nc()
    nc = _CACHE["nc"]
    in_maps = _prep_in_maps(x, wq, bq, wk, bk, wv, bv, wo)
    res = run_bass_kernel_spmd(nc, in_maps, list(range(NCORES)),
                               **(_run_kwargs or {}))
    acc = np.zeros((NT, D), np.float32)
    for c in range(NCORES):
        acc += res.results[c]["out"].astype(np.float32)
    acc += np.asarray(bo, np.float32)[None, :]
    if _run_kwargs:
        _CACHE["last_results"] = res
    return acc.reshape(B, S, D)

